# revision 22
# baseline (speedup 1.0000x reference)
"""Trainium2 Bass kernel: GQA attention layer (nn_Attention), tensor-parallel
over heads across 8 NeuronCores.

Sharding (TP8): core c owns kv head c and q heads 4c..4c+3 (GQA groups are
contiguous), i.e. rows [512c, 512c+512) of Wq, rows [128c, 128c+128) of
Wk/Wv, and columns [512c, 512c+512) of Wo.  Each core computes a full
[HID, TOK] partial of the output; the host sums the 8 partials (all-reduce)
and transposes back to [B, S, HID].

All on-device matmuls run as float32r (FP22-truncated fp32, full PE rate).
Everything is computed in a transposed layout (feature-on-partition,
token-on-free) so no on-chip transposes of activations are needed:
  Q^T = Wq^T.T @ X^T    (lhsT = Wq^T tile, rhs = X^T tile)
  S^T = K'^T.T @ Q'^T   -> exp on ACT -> P^T
  O^T[dv,t] = sum_key V[key,dv] P^T[key,t]   (lhsT = V tile, rhs = P^T)
  Y^T = Wo^T.T @ O'^T
Softmax denominators come from a ones-vector matmul accumulated alongside AV;
normalization is applied after AV (flash-attention style).  RoPE's
rotate-half is a 2-descriptor SBUF->SBUF DMA (partition swap) with the sign
baked into the host-provided SIN table.
"""

import math
from contextlib import ExitStack, nullcontext

import numpy as np

import concourse.bass as bass
import concourse.tile as tile
from concourse import bacc
from concourse import mybir
from concourse.bass import ts

# Problem constants (hardcoded; kernel.py must be self-contained).
HIDDEN = 4096
N_HEADS = 32
N_KV_HEADS = 8
D = 128                      # head dim
B = 2
S = 2048
N_CORES = 8
QH = N_HEADS // N_CORES      # q heads per core = 4
ROPE_THETA = 10000.0
SCALE = 1.0 / math.sqrt(D)

F32 = mybir.dt.float32
F32R = mybir.dt.float32r
EXP = mybir.ActivationFunctionType.Exp


def build_nc(hid=HIDDEN, s=S, b=B, qh=QH, pb=256, ab=512, timing_loop=None, tune=None):
    """Build the per-core Bass program (same SPMD program on all cores).

    timing_loop: if set, big I/O becomes Internal (no host transfer) and the
    whole body runs `timing_loop` times inside a Tile For_i so the kernel
    duration can be measured by differential wall-clock.
    """
    tn = dict(x=2, st=8, pss=2, pso=2, psden=1, psrb=1, psv=2, pT=3,
              qpool=2, rot=1, ropetmp=1, sm=2, y=6, psy=3,
              no_den=False, skip_p2=False, skip_p3=False,
              gpb=True, pair=True, no_rope=False, no_av=False, no_norm=False,
              dsp=1, p1db=0)
    if tune:
        tn.update(tune)
    dsp = tn["dsp"]
    tok = b * s
    _sk = s // 128
    den_dve = (_sk // 2) > dsp       # any pairs accumulated on DVE?
    last_pe_den = min(_sk, 2 * dsp) - 1
    kt_n = hid // 128            # contraction tiles for projections
    npb = tok // pb              # phase-1 token blocks
    nab = s // ab                # attention token blocks per batch
    sk = s // 128                # key tiles per batch
    ot_n = qh + 2                # projection out-tiles: qh q-heads + K + V
    qdim = qh * 128

    nc = bacc.Bacc("TRN2", target_bir_lowering=False, debug=False)

    big = "Internal" if timing_loop else "ExternalInput"
    xt = nc.dram_tensor("xt", [npb, 128, kt_n, pb], F32R, kind=big)
    wqt = nc.dram_tensor("wqt", [128, kt_n, qdim], F32R, kind=big)
    wkt = nc.dram_tensor("wkt", [128, kt_n, 128], F32R, kind=big)
    wvt = nc.dram_tensor("wvt", [128, kt_n, 128], F32R, kind=big)
    wot = nc.dram_tensor("wot", [qh, 128, hid], F32R, kind=big)
    cos_d = nc.dram_tensor("cos_t", [128, s], F32R, kind="ExternalInput")
    sin_d = nc.dram_tensor("sin_t", [128, s], F32R, kind="ExternalInput")  # sign-baked
    ident_d = nc.dram_tensor("ident", [128, 128], F32R, kind="ExternalInput")
    ones_d = nc.dram_tensor("ones", [128, 128], F32R, kind="ExternalInput")
    if timing_loop:
        yt = nc.dram_tensor("yt", [hid, tok], F32, kind="Internal")
        yt_small = nc.dram_tensor("yt_small", [128, 128], F32, kind="ExternalOutput")
    else:
        yt = nc.dram_tensor("yt", [hid, tok], F32, kind="ExternalOutput")

    with tile.TileContext(nc) as tc, ExitStack() as top:
        # DRAM scratch for the QKV roundtrip (SBUF can't hold W + X + QKV).
        dpool = top.enter_context(tc.tile_pool(name="dram", bufs=1, space="DRAM"))
        qt_d = dpool.tile([qh, 128, tok], F32R, name="qt_d")
        kt_d = dpool.tile([128, tok], F32R, name="kt_d")
        vt_d = dpool.tile([128, tok], F32R, name="vt_d")

        if timing_loop:
            # Zero-fill internal inputs once so the timed loop sees sane data.
            with tc.tile_pool(name="zero", bufs=1) as zp:
                zt = zp.tile([128, kt_n * qdim], F32, name="zt")
                nc.gpsimd.memset(zt[:], 0.0)
                zr = zt[:].bitcast(F32R)
                for tb in range(npb):
                    nc.sync.dma_start(
                        xt[tb],
                        zr[:, : kt_n * pb].rearrange("p (a c) -> p a c", a=kt_n),
                    )
                nc.sync.dma_start(
                    wqt[:],
                    zr[:, : kt_n * qdim].rearrange("p (a c) -> p a c", a=kt_n))
                nc.sync.dma_start(
                    wkt[:],
                    zr[:, : kt_n * 128].rearrange("p (a c) -> p a c", a=kt_n))
                nc.sync.dma_start(
                    wvt[:],
                    zr[:, : kt_n * 128].rearrange("p (a c) -> p a c", a=kt_n))
                for dv in range(qh):
                    nc.sync.dma_start(wot[dv], zr[:, :hid])

        loop_cm = tc.For_i(0, timing_loop, 1) if timing_loop else nullcontext()
        with loop_cm:
            # ------------- Phase 1: QKV projections (transposed outputs) ----
            with ExitStack() as p1:
                wpool = p1.enter_context(tc.tile_pool(name="p1w", bufs=1))
                xpool = p1.enter_context(tc.tile_pool(name="p1x", bufs=tn["x"]))
                stpool = p1.enter_context(tc.tile_pool(name="p1st", bufs=tn["st"]))
                pspool = p1.enter_context(
                    tc.tile_pool(name="p1ps", bufs=1, space="PSUM"))

                wq_sb = wpool.tile([128, kt_n, qdim], F32R, name="wq_sb")
                for kt in range(kt_n):
                    nc.sync.dma_start(wq_sb[:, kt, :], wqt[:, kt, :])
                wk_sb = wpool.tile([128, kt_n, 128], F32R, name="wk_sb")
                wv_sb = wpool.tile([128, kt_n, 128], F32R, name="wv_sb")
                wchunk = min(8, kt_n)
                for c0 in range(0, kt_n, wchunk):
                    nc.sync.dma_start(wk_sb[:, c0:c0 + wchunk, :],
                                      wkt[:, c0:c0 + wchunk, :])
                    nc.sync.dma_start(wv_sb[:, c0:c0 + wchunk, :],
                                      wvt[:, c0:c0 + wchunk, :])

                for tb in range(npb):
                    x_sb = xpool.tile([128, kt_n, pb], F32R, tag="x", name="x_sb")
                    nc.sync.dma_start(x_sb[:], xt[tb])
                    for ot in range(ot_n):
                        ps = pspool.tile([128, pb], F32, tag=f"ps{ot}",
                                         bufs=(2 if ot < tn["p1db"] else 1),
                                         name="ps1")
                        for kt in range(kt_n):
                            if ot < qh:
                                w = wq_sb[:, kt, ts(ot, 128)]
                            elif ot == qh:
                                w = wk_sb[:, kt, :]
                            else:
                                w = wv_sb[:, kt, :]
                            nc.tensor.matmul(
                                ps[:], w, x_sb[:, kt, :],
                                start=(kt == 0), stop=(kt == kt_n - 1),
                            )
                        st = stpool.tile([128, pb], F32R, tag="st", name="st")
                        nc.scalar.copy(st[:], ps[:])
                        if ot < qh:
                            dst = qt_d[ot, :, ts(tb, pb)]
                        elif ot == qh:
                            dst = kt_d[:, ts(tb, pb)]
                        else:
                            dst = vt_d[:, ts(tb, pb)]
                        nc.sync.dma_start(dst, st[:])

            with ExitStack() as rest:
                # ------------- Constants (phase 2/3) ----------------------
                cpool = rest.enter_context(tc.tile_pool(name="consts", bufs=1))
                ident = cpool.tile([128, 128], F32R, name="ident")
                nc.sync.dma_start(ident[:], ident_d.ap())
                ones_sb = cpool.tile([128, 128], F32R, name="ones_sb")
                nc.sync.dma_start(ones_sb[:], ones_d.ap())
                ones_col = ones_sb[:, 0:1]
                ones_row = ones_sb[0:1, :]
                cos_sb = cpool.tile([128, s], F32R, name="cos_sb")
                nc.sync.dma_start(cos_sb[:], cos_d.ap())
                sin_sb = cpool.tile([128, s], F32R, name="sin_sb")
                nc.sync.dma_start(sin_sb[:], sin_d.ap())

                def rope(src, dst, tpool):
                    """dst = src*cos + rot_half(src)*sin (sign baked in sin).

                    Cross-partition move via SBUF->SBUF DMA (DVE can't)."""
                    rot = tpool.tile([128, s], F32R, tag="rot", bufs=tn["rot"],
                                     name="rot")
                    nc.sync.dma_start(rot[0:64, :], src[64:128, :])
                    nc.sync.dma_start(rot[64:128, :], src[0:64, :])
                    t1 = tpool.tile([128, s], F32R, tag="ropetmp", bufs=tn["ropetmp"],
                                    name="ropetmp")
                    nc.vector.tensor_mul(t1[:], src, cos_sb[:])
                    nc.vector.tensor_mul(rot[:], rot[:], sin_sb[:])
                    nc.vector.tensor_add(dst, t1[:], rot[:])

                # Persistent per-head attention outputs O'^T [d, tok].
                opool = rest.enter_context(tc.tile_pool(name="oT", bufs=1))
                oT = [opool.tile([128, tok], F32R, name=f"oT{h}")
                      for h in range(qh)]

                # ------------- Phase 2: attention -------------------------
                with ExitStack() as p2:
                    tpool = p2.enter_context(tc.tile_pool(name="p2t", bufs=1))
                    qpool = p2.enter_context(tc.tile_pool(name="p2q", bufs=tn["qpool"]))
                    ppool = p2.enter_context(tc.tile_pool(name="p2p", bufs=tn["pT"]))
                    smpool = p2.enter_context(tc.tile_pool(name="p2sm", bufs=tn["sm"]))
                    ps2 = p2.enter_context(
                        tc.tile_pool(name="p2ps", bufs=1, space="PSUM"))

                    for bb in range(b) if not tn["skip_p2"] else []:
                        # K^T for this batch + RoPE.
                        k_raw = tpool.tile([128, s], F32R, tag="kraw",
                                           name="k_raw")
                        nc.sync.dma_start(k_raw[:], kt_d[:, ts(bb, s)])
                        kT = tpool.tile([128, s], F32R, tag="kT", name="kT")
                        rope(k_raw[:], kT[:], tpool)

                        # V natural [key, dv] via identity matmuls from V^T.
                        v_raw = tpool.tile([128, s], F32R, tag="vraw",
                                           name="v_raw")
                        nc.sync.dma_start(v_raw[:], vt_d[:, ts(bb, s)])
                        v_sb = tpool.tile([128, s], F32R, tag="vsb", name="v_sb")
                        for k2 in range(sk):
                            psv = ps2.tile(
                                [128, 128], F32,
                                tag="pss" if tn["pair"] else "psv",
                                bufs=tn["pss"] if tn["pair"] else tn["psv"],
                                name="psv")
                            nc.tensor.matmul(
                                psv[:], v_raw[:, ts(k2, 128)], ident[:],
                                start=True, stop=True,
                            )
                            nc.scalar.copy(v_sb[:, ts(k2, 128)], psv[:])

                        for h in range(qh):
                            q_raw = qpool.tile([128, s], F32R, tag="qraw",
                                               name="q_raw")
                            nc.sync.dma_start(q_raw[:], qt_d[h, :, ts(bb, s)])
                            if tn["no_rope"]:
                                qT = q_raw
                            else:
                                qT = qpool.tile([128, s], F32R, tag="qT", name="qT")
                                rope(q_raw[:], qT[:], qpool)

                            for a in range(nab):
                                ps_o = ps2.tile([128, ab], F32, tag="pso",
                                                bufs=tn["pso"], name="ps_o")
                                ps_den = ps2.tile([1, ab], F32, tag="psden",
                                                  bufs=tn["psden"], name="ps_den")
                                for jp in range(sk // 2):
                                    ps_s = ps2.tile([128, 2 * ab], F32, tag="pss",
                                                    bufs=tn["pss"], name="ps_s")
                                    for u in (0, 1):
                                        k2 = 2 * jp + u
                                        nc.tensor.matmul(
                                            ps_s[:, ts(u, ab)], kT[:, ts(k2, 128)],
                                            qT[:, ts(a, ab)],
                                            start=True, stop=True,
                                        )
                                    pT = ppool.tile([128, 2 * ab], F32R, tag="pT",
                                                    name="pT")
                                    nc.scalar.activation(pT[:], ps_s[:], EXP,
                                                         scale=SCALE)
                                    for u in (0, 1):
                                        k2 = 2 * jp + u
                                        if not tn["no_av"]:
                                            nc.tensor.matmul(
                                                ps_o[:], v_sb[:, ts(k2, 128)],
                                                pT[:, ts(u, ab)],
                                                start=(k2 == 0), stop=(k2 == sk - 1),
                                            )
                                        # Denominator: first `dsp` pairs via PE
                                        # ones-matmul; the rest accumulate on
                                        # DVE (balances PE vs ACT vs DVE).
                                        if not tn["no_den"] and jp < dsp:
                                            nc.tensor.matmul(
                                                ps_den[:], ones_col,
                                                pT[:, ts(u, ab)],
                                                start=(k2 == 0),
                                                stop=(not den_dve
                                                      and k2 == last_pe_den),
                                            )
                                    if not tn["no_den"] and jp >= dsp:
                                        if jp == dsp:
                                            dacc = smpool.tile(
                                                [128, ab], F32R, tag="dacc",
                                                name="dacc")
                                            nc.vector.tensor_add(
                                                dacc[:], pT[:, 0:ab], pT[:, ab:2 * ab])
                                        else:
                                            dtmp = smpool.tile(
                                                [128, ab], F32R, tag="dtmp",
                                                name="dtmp")
                                            nc.vector.tensor_add(
                                                dtmp[:], pT[:, 0:ab], pT[:, ab:2 * ab])
                                            nc.vector.tensor_add(
                                                dacc[:], dacc[:], dtmp[:])
                                if not tn["no_den"] and den_dve:
                                    nc.tensor.matmul(
                                        ps_den[:], ones_col, dacc[:],
                                        start=False, stop=True,
                                    )
                                rcp = smpool.tile([1, ab], F32R, tag="rcp",
                                                  name="rcp")
                                if not tn["no_den"]:
                                    with nc.allow_low_precision(
                                            reason="f32r softmax denominators"):
                                        nc.vector.reciprocal(rcp[:], ps_den[:])
                                else:
                                    with nc.allow_low_precision(reason="x"):
                                        nc.vector.reciprocal(rcp[:], cos_sb[0:1, 0:ab])
                                rb = smpool.tile([128, ab], F32R, tag="rb",
                                                 name="rb")
                                if tn["gpb"]:
                                    nc.gpsimd.partition_broadcast(rb[:], rcp[:])
                                else:
                                    ps_rb = ps2.tile([128, ab], F32, tag="psrb",
                                                     bufs=tn["psrb"], name="ps_rb")
                                    nc.tensor.matmul(
                                        ps_rb[:], ones_row, rcp[:],
                                        start=True, stop=True,
                                    )
                                    nc.scalar.copy(rb[:], ps_rb[:])
                                if not tn["no_norm"]:
                                    nc.vector.tensor_mul(
                                        oT[h][:, bass.ds(bb * s + a * ab, ab)],
                                        ps_o[:], rb[:]
                                    )

                # ------------- Phase 3: output projection -----------------
                with ExitStack() as p3:
                    w3pool = p3.enter_context(tc.tile_pool(name="p3w", bufs=1))
                    ypool = p3.enter_context(tc.tile_pool(name="p3y", bufs=tn["y"]))
                    ps3 = p3.enter_context(
                        tc.tile_pool(name="p3ps", bufs=tn["psy"], space="PSUM"))

                    wo_sb = []
                    for dv in range(qh):
                        w = w3pool.tile([128, hid], F32R, name=f"wo_sb{dv}")
                        nc.sync.dma_start(w[:], wot[dv])
                        wo_sb.append(w)

                    for tb3 in range(tok // ab) if not tn["skip_p3"] else []:
                        for ht in range(hid // 128):
                            ps_y = ps3.tile([128, ab], F32, tag="psy",
                                            name="ps_y")
                            for dv in range(qh):
                                nc.tensor.matmul(
                                    ps_y[:], wo_sb[dv][:, ts(ht, 128)],
                                    oT[dv][:, ts(tb3, ab)],
                                    start=(dv == 0), stop=(dv == qh - 1),
                                )
                            y_sb = ypool.tile([128, ab], F32, tag="y",
                                              name="y_sb")
                            nc.scalar.copy(y_sb[:], ps_y[:])
                            nc.sync.dma_start(
                                yt.ap()[ts(ht, 128), ts(tb3, ab)], y_sb[:])

        if timing_loop:
            with tc.tile_pool(name="smallout", bufs=1) as sp:
                t = sp.tile([128, 128], F32, name="t_small")
                nc.sync.dma_start(t[:], yt.ap()[0:128, 0:128])
                nc.sync.dma_start(yt_small.ap()[:, :], t[:])

    nc.compile()
    return nc


# ----------------------------------------------------------------------------
# Host side
# ----------------------------------------------------------------------------

def _rope_tables(position_ids, s):
    """cos^T/sin^T tables [128, s] in d-on-partition layout; sin sign-baked."""
    pos = np.asarray(position_ids).reshape(-1).astype(np.float64)
    assert pos.shape[0] == s
    inv = 1.0 / (ROPE_THETA ** (np.arange(0, D, 2, dtype=np.float64) / D))  # [64]
    f = inv[:, None] * pos[None, :]                      # [64, s]
    ff = np.concatenate([f, f], axis=0)                  # [128, s]
    cos_t = np.cos(ff).astype(np.float32)
    sin_t = np.sin(ff).astype(np.float32)
    sin_t[:64] *= -1.0                                   # rot[0:64] = -q[64:128]
    return np.ascontiguousarray(cos_t), np.ascontiguousarray(sin_t)


def _prep_in_maps(hidden_states, position_ids, Wq, Wk, Wv, Wo,
                  hid=HIDDEN, s=S, b=B, qh=QH, pb=256, n_cores=N_CORES):
    tok = b * s
    kt_n = hid // 128
    npb = tok // pb
    qdim = qh * 128

    X = np.ascontiguousarray(
        np.asarray(hidden_states, dtype=np.float32).reshape(tok, hid))
    # xt[tb, p, kt, t] = X[tb*pb + t, kt*128 + p]
    xt = np.ascontiguousarray(X.reshape(npb, pb, kt_n, 128).transpose(0, 3, 2, 1))
    cos_t, sin_t = _rope_tables(position_ids, s)

    Wq = np.asarray(Wq, dtype=np.float32)
    Wk = np.asarray(Wk, dtype=np.float32)
    Wv = np.asarray(Wv, dtype=np.float32)
    Wo = np.asarray(Wo, dtype=np.float32)

    maps = []
    for c in range(n_cores):
        wq = Wq[c * qdim:(c + 1) * qdim].T                 # [hid, qdim]
        wqt = np.ascontiguousarray(wq.reshape(kt_n, 128, qdim).transpose(1, 0, 2))
        wk = Wk[c * 128:(c + 1) * 128].T
        wkt = np.ascontiguousarray(wk.reshape(kt_n, 128, 128).transpose(1, 0, 2))
        wv = Wv[c * 128:(c + 1) * 128].T
        wvt = np.ascontiguousarray(wv.reshape(kt_n, 128, 128).transpose(1, 0, 2))
        wo = np.ascontiguousarray(Wo[:, c * qdim:(c + 1) * qdim].T)  # [qdim, hid]
        wot = wo.reshape(qh, 128, hid)
        maps.append({
            "xt": xt, "wqt": wqt, "wkt": wkt, "wvt": wvt, "wot": wot,
            "cos_t": cos_t, "sin_t": sin_t,
            "ident": np.eye(128, dtype=np.float32),
            "ones": np.ones((128, 128), dtype=np.float32),
        })
    return maps


_NC_CACHE = {}


def _get_nc():
    if "nc" not in _NC_CACHE:
        _NC_CACHE["nc"] = build_nc()
    return _NC_CACHE["nc"]


def run(inputs, trace=False, **kw):
    """Run the SPMD kernel on 8 cores; returns (full_output, BassKernelResults)."""
    from concourse import bass_utils
    in_maps = _prep_in_maps(
        inputs["hidden_states"], inputs["position_ids"],
        inputs["Wq"], inputs["Wk"], inputs["Wv"], inputs["Wo"],
    )
    nc = _get_nc()
    res = bass_utils.run_bass_kernel_spmd(
        nc, in_maps, core_ids=list(range(N_CORES)), trace=trace, **kw
    )
    acc = np.zeros((HIDDEN, B * S), dtype=np.float64)
    for r_ in res.results:
        acc += r_["yt"]
    out = np.ascontiguousarray(acc.T.astype(np.float32).reshape(B, S, HIDDEN))
    return out, res


def kernel(**inputs) -> np.ndarray:
    out, _ = run(inputs, trace=False)
    return out
